# revision 1
# baseline (speedup 1.0000x reference)
"""Trainium2 Bass kernel for nn_KeypointLoss: data-parallel over batch (8 cores).

Per core (4 samples): streams hm_preds (23MB) + heatmaps (11.5MB) from HBM;
label predictions are fetched with an indirect (gather) DMA at the 44 argmax
locations instead of streaming the full 14.7MB tensor.

Argmax scheme (exact, first-occurrence tie-break like jnp.argmax):
 - colmax[p, j]  = max over f of gt[j][p, f]           (segmented reduce)
 - rowsum[p, j]  = sum_f (gt >= colmax) * (512 - f)    (one fused STT op per
   image; equals 512 - argmax_f for rows with a unique row max)
 - transpose both to [44, 128]; global max -> select lowest tied partition p*
   via max of mask*(128-p); pick that row's rowsum via a one-hot; combine to
   the flat pixel index; indirect-DMA gather lb_preds at those 44 locations.
"""
import sys
import numpy as np

sys.path.insert(0, "/opt/trn_rl_repo")

import concourse.bacc as bacc
import concourse.mybir as mybir
import concourse.tile as tile
from concourse.bass import IndirectOffsetOnAxis
from concourse.bass_utils import run_bass_kernel_spmd

F32 = mybir.dt.float32
I32 = mybir.dt.int32

B_LOC = 4      # batch per core
S = 2          # stacks
K = 11         # keypoints
C = 7          # label channels
HW = 65536     # 256*256
P = 128        # partitions
FK = HW // P   # 512
NJ = B_LOC * K  # 44 (b,k) images per core
NSC = S * C     # 14 (s,c) pairs
NHALF = 2       # split each (b,s) pred pass for SBUF headroom
FH = FK // NHALF

_CACHE = {}


def _consts():
    negp = np.broadcast_to((P - np.arange(P, dtype=np.float32))[None, :], (NJ, P)).copy()
    negf = np.broadcast_to((FK - np.arange(FK, dtype=np.float32))[None, :], (P, FK)).copy()
    b_of_j = np.arange(NJ) // K
    sc = (np.arange(S)[:, None] * C + np.arange(C)[None, :]).reshape(-1)
    base = (b_of_j[:, None] * S * C + sc[None, :]).astype(np.float32) * HW
    ones = np.ones((P, 1), np.float32)
    blockind = (b_of_j[:, None] == np.arange(B_LOC)[None, :]).astype(np.float32)
    ident = np.eye(P, dtype=np.float32)
    return dict(negp=negp, negf=negf, base=base, ones=ones, blockind=blockind,
                ident=ident)


def _build(reps=1, mode='full'):
    nc = bacc.Bacc("TRN2", target_bir_lowering=False, debug=False,
                   enable_asserts=False, num_devices=8)
    hm = nc.dram_tensor("hm", [B_LOC, S, K, HW], F32, kind="ExternalInput").ap()
    gt = nc.dram_tensor("gt", [B_LOC, K, HW], F32, kind="ExternalInput").ap()
    lb = nc.dram_tensor("lb", [B_LOC * S * C * HW, 1], F32, kind="ExternalInput").ap()
    labels_bc = nc.dram_tensor("labels_bc", [NJ, NSC], F32, kind="ExternalInput").ap()
    negp_d = nc.dram_tensor("negp", [NJ, P], F32, kind="ExternalInput").ap()
    negf_d = nc.dram_tensor("negf", [P, FK], F32, kind="ExternalInput").ap()
    base_d = nc.dram_tensor("base", [NJ, NSC], F32, kind="ExternalInput").ap()
    ones_d = nc.dram_tensor("ones", [P, 1], F32, kind="ExternalInput").ap()
    blk_d = nc.dram_tensor("blockind", [NJ, B_LOC], F32, kind="ExternalInput").ap()
    id_d = nc.dram_tensor("ident", [P, P], F32, kind="ExternalInput").ap()
    hm_out = nc.dram_tensor("hm_out", [1, B_LOC * S * NHALF], F32,
                            kind="ExternalOutput").ap()
    lb_out = nc.dram_tensor("lb_out", [B_LOC, S], F32, kind="ExternalOutput").ap()
    dbg_flat = nc.dram_tensor("dbg_flat", [NJ, 1], F32, kind="ExternalOutput").ap()
    dbg_gath = nc.dram_tensor("dbg_gath", [NJ, NSC], F32, kind="ExternalOutput").ap()

    with tile.TileContext(nc) as tc:
        with (
            tc.tile_pool(name="gtp", bufs=B_LOC) as gtp,
            tc.tile_pool(name="work", bufs=4) as work,
            tc.tile_pool(name="work2", bufs=2) as work2,
            tc.tile_pool(name="small", bufs=1) as small,
            tc.tile_pool(name="psum", bufs=1, space="PSUM") as psp,
        ):
            negp_t = small.tile([NJ, P], F32, tag="negp")
            negf_t = small.tile([P, FK], F32, tag="negf")
            base_t = small.tile([NJ, NSC], F32, tag="base")
            ones_t = small.tile([P, 1], F32, tag="ones")
            blk_t = small.tile([NJ, B_LOC], F32, tag="blk")
            id_t = small.tile([P, P], F32, tag="ident")
            lab_t = small.tile([NJ, NSC], F32, tag="lab")
            for t, d in ((negp_t, negp_d), (negf_t, negf_d), (base_t, base_d),
                         (ones_t, ones_d), (blk_t, blk_d), (id_t, id_d),
                         (lab_t, labels_bc)):
                nc.sync.dma_start(out=t[:], in_=d)

            for _rep in range(reps):
                colmax = small.tile([P, NJ], F32, tag="colmax")
                rowsum = small.tile([P, NJ], F32, tag="rowsum")
                acc = small.tile([P, B_LOC * S * NHALF], F32, tag="acc")

                # Phase 1: all gt loads + argmax per-image work up front so
                # the argmax->gather->label tail hides under pred streaming.
                gt3s = []
                for b in range(B_LOC):
                    gt_t = gtp.tile([P, K * FK], F32, tag="gt")
                    nc.sync.dma_start(
                        out=gt_t[:].rearrange("p (k f) -> p k f", k=K),
                        in_=gt[b].rearrange("k (p f) -> p k f", p=P),
                    )
                    gt3 = gt_t[:].rearrange("p (k f) -> p k f", k=K)
                    gt3s.append(gt3)
                    if mode in ('hm', 'dma'):
                        continue
                    nc.vector.tensor_reduce(
                        out=colmax[:, b * K:(b + 1) * K], in_=gt3,
                        axis=mybir.AxisListType.X, op=mybir.AluOpType.max,
                    )
                    # fused per-image row argmax: rowsum = sum((gt>=colmax)*(512-f))
                    for k in range(K):
                        j = b * K + k
                        msk_t = work2.tile([P, FK], F32, tag="msk")
                        nc.vector.scalar_tensor_tensor(
                            out=msk_t[:], in0=gt3[:, k, :],
                            scalar=colmax[:, j:j + 1], in1=negf_t[:],
                            op0=mybir.AluOpType.is_ge, op1=mybir.AluOpType.mult,
                            accum_out=rowsum[:, j:j + 1],
                        )
                # Phase 2: stream preds for the heatmap loss.
                for b in range(B_LOC):
                    for s in range(S):
                        for h in range(NHALF):
                            pred_t = work.tile([P, K * FH], F32, tag="pred")
                            nc.sync.dma_start(
                                out=pred_t[:].rearrange("p (k f) -> p k f", k=K),
                                in_=hm[b, s].rearrange("k (p f) -> p k f", p=P)[
                                    :, :, h * FH:(h + 1) * FH],
                            )
                            if mode == 'dma':
                                continue
                            diff_t = work2.tile([P, K * FH], F32, tag="diff")
                            nc.vector.tensor_tensor(
                                out=diff_t[:],
                                in0=pred_t[:],
                                in1=gt3s[b][:, :, h * FH:(h + 1) * FH],
                                op=mybir.AluOpType.subtract,
                            )
                            col = (b * S + s) * NHALF + h
                            nc.scalar.activation(
                                out=pred_t[:], in_=diff_t[:],
                                func=mybir.ActivationFunctionType.Square,
                                accum_out=acc[:, col:col + 1],
                            )

                # ---- argmax combine stage (all tiny [44,x] ops) ----
                skip_tail = mode in ('hm', 'dma')
                if not skip_tail:
                    cm_p = psp.tile([NJ, P], F32, tag="cmp", space="PSUM")
                    nc.tensor.transpose(out=cm_p[:], in_=colmax[:], identity=id_t[:])
                    cmT = small.tile([NJ, P], F32, tag="cmT")
                    nc.vector.tensor_copy(out=cmT[:], in_=cm_p[:])
                    rs_p = psp.tile([NJ, P], F32, tag="rsp", space="PSUM")
                    nc.tensor.transpose(out=rs_p[:], in_=rowsum[:], identity=id_t[:])
                    rsT = small.tile([NJ, P], F32, tag="rsT")
                    nc.vector.tensor_copy(out=rsT[:], in_=rs_p[:])

                    gmax = small.tile([NJ, 1], F32, tag="gmax")
                    nc.vector.tensor_reduce(out=gmax[:], in_=cmT[:],
                                            axis=mybir.AxisListType.X,
                                            op=mybir.AluOpType.max)
                    maskT = small.tile([NJ, P], F32, tag="maskT")
                    nc.vector.tensor_scalar(out=maskT[:], in0=cmT[:], scalar1=gmax[:],
                                            scalar2=None, op0=mybir.AluOpType.is_ge)
                    scoreT = small.tile([NJ, P], F32, tag="scoreT")
                    nc.vector.tensor_tensor(out=scoreT[:], in0=maskT[:], in1=negp_t[:],
                                            op=mybir.AluOpType.mult)
                    pscore = small.tile([NJ, 1], F32, tag="pscore")
                    nc.vector.tensor_reduce(out=pscore[:], in_=scoreT[:],
                                            axis=mybir.AxisListType.X,
                                            op=mybir.AluOpType.max)
                    onehotT = small.tile([NJ, P], F32, tag="onehotT")
                    nc.vector.tensor_scalar(out=onehotT[:], in0=negp_t[:],
                                            scalar1=pscore[:], scalar2=None,
                                            op0=mybir.AluOpType.is_equal)
                    fsel = small.tile([NJ, P], F32, tag="fsel")
                    nc.vector.tensor_tensor(out=fsel[:], in0=onehotT[:], in1=rsT[:],
                                            op=mybir.AluOpType.mult)
                    fscore = small.tile([NJ, 1], F32, tag="fscore")
                    nc.vector.tensor_reduce(out=fscore[:], in_=fsel[:],
                                            axis=mybir.AxisListType.X,
                                            op=mybir.AluOpType.max)
                    # flat = (128-pscore)*512 + (512-fscore)
                    t1 = small.tile([NJ, 1], F32, tag="t1")
                    nc.vector.tensor_scalar(out=t1[:], in0=pscore[:], scalar1=-512.0,
                                            scalar2=None, op0=mybir.AluOpType.mult)
                    flatf = small.tile([NJ, 1], F32, tag="flatf")
                    nc.vector.scalar_tensor_tensor(
                        out=flatf[:], in0=t1[:], scalar=float(P * FK + FK),
                        in1=fscore[:], op0=mybir.AluOpType.add,
                        op1=mybir.AluOpType.subtract,
                    )
                    off_f = small.tile([NJ, NSC], F32, tag="off_f")
                    nc.vector.tensor_scalar(out=off_f[:], in0=base_t[:],
                                            scalar1=flatf[:], scalar2=None,
                                            op0=mybir.AluOpType.add)
                    off_i = small.tile([NJ, NSC], I32, tag="off_i")
                    nc.vector.tensor_copy(out=off_i[:], in_=off_f[:])

                    gath = small.tile([NJ, NSC], F32, tag="gath")
                    if mode == 'noga':
                        nc.sync.dma_start(out=gath[:], in_=base_d)
                    else:
                        for sc in range(NSC):
                            nc.gpsimd.indirect_dma_start(
                                out=gath[:, sc:sc + 1], out_offset=None, in_=lb,
                                in_offset=IndirectOffsetOnAxis(
                                    ap=off_i[:, sc:sc + 1], axis=0),
                            )

                    nc.sync.dma_start(out=dbg_flat, in_=flatf[:])
                    nc.sync.dma_start(out=dbg_gath, in_=gath[:])

                    ldiff = small.tile([NJ, NSC], F32, tag="ldiff")
                    nc.vector.tensor_tensor(out=ldiff[:], in0=gath[:], in1=lab_t[:],
                                            op=mybir.AluOpType.subtract)
                    lsq = small.tile([NJ, NSC], F32, tag="lsq")
                    nc.scalar.activation(out=lsq[:], in_=ldiff[:],
                                         func=mybir.ActivationFunctionType.Square)
                    persum = small.tile([NJ, S], F32, tag="persum")
                    nc.vector.tensor_reduce(
                        out=persum[:],
                        in_=lsq[:].rearrange("j (s c) -> j s c", s=S),
                        axis=mybir.AxisListType.X, op=mybir.AluOpType.add)
                    lb_p = psp.tile([B_LOC, S], F32, tag="lbp", space="PSUM")
                    nc.tensor.matmul(out=lb_p[:], lhsT=blk_t[:], rhs=persum[:],
                                     start=True, stop=True)
                    lb_s = small.tile([B_LOC, S], F32, tag="lbs")
                    nc.scalar.activation(out=lb_s[:], in_=lb_p[:],
                                         func=mybir.ActivationFunctionType.Copy,
                                         scale=1.0 / (K * C))
                    nc.sync.dma_start(out=lb_out, in_=lb_s[:])

                hm_p = psp.tile([1, B_LOC * S * NHALF], F32, tag="hmp", space="PSUM")
                nc.tensor.matmul(out=hm_p[:], lhsT=ones_t[:], rhs=acc[:],
                                 start=True, stop=True)
                hm_s = small.tile([1, B_LOC * S * NHALF], F32, tag="hms")
                nc.scalar.activation(out=hm_s[:], in_=hm_p[:],
                                     func=mybir.ActivationFunctionType.Copy,
                                     scale=1.0 / (K * HW))
                nc.sync.dma_start(out=hm_out, in_=hm_s[:])

    nc.compile()
    return nc


def _get_nc():
    if "nc" not in _CACHE:
        _CACHE["nc"] = _build()
    return _CACHE["nc"]


def make_in_maps(combined_hm_preds, combined_lb_preds, heatmaps, labels):
    consts = _consts()
    in_maps = []
    for c in range(8):
        sl = slice(c * B_LOC, (c + 1) * B_LOC)
        lab = np.asarray(labels[sl], np.float32)  # [4, 11, 7]
        lab_bc = np.broadcast_to(
            lab[:, :, None, :], (B_LOC, K, S, C)).reshape(NJ, NSC)
        m = {
            "hm": np.ascontiguousarray(
                np.asarray(combined_hm_preds[sl], np.float32).reshape(
                    B_LOC, S, K, HW)),
            "gt": np.ascontiguousarray(
                np.asarray(heatmaps[sl], np.float32).reshape(B_LOC, K, HW)),
            "lb": np.ascontiguousarray(
                np.asarray(combined_lb_preds[sl], np.float32).reshape(
                    B_LOC * S * C * HW, 1)),
            "labels_bc": np.ascontiguousarray(lab_bc),
        }
        m.update(consts)
        in_maps.append(m)
    return in_maps


def run(in_maps, trace=False, **kw):
    nc = _get_nc()
    return run_bass_kernel_spmd(nc, in_maps, list(range(8)), trace=trace, **kw)


def make_pjrt_runner(nc, in_maps):
    """Device-resident repeat runner (mimics bass2jax.run_bass_via_pjrt)."""
    import jax
    from jax.experimental.shard_map import shard_map
    from jax.sharding import Mesh, NamedSharding, PartitionSpec
    from concourse.bass2jax import (_bass_exec_p, install_neuronx_cc_hook,
                                    partition_id_tensor)
    install_neuronx_cc_hook()
    n_cores = len(in_maps)
    partition_name = (nc.partition_id_tensor.name
                      if nc.partition_id_tensor else None)
    in_names, out_names, out_avals, zero_outs = [], [], [], []
    for alloc in nc.m.functions[0].allocations:
        if not isinstance(alloc, mybir.MemoryLocationSet):
            continue
        name = alloc.memorylocations[0].name
        if alloc.kind == "ExternalInput":
            if name != partition_name:
                in_names.append(name)
        elif alloc.kind == "ExternalOutput":
            shape = tuple(alloc.tensor_shape)
            dtype = mybir.dt.np(alloc.dtype)
            out_names.append(name)
            out_avals.append(jax.core.ShapedArray(shape, dtype))
            zero_outs.append(np.zeros(shape, dtype))
    n_params, n_outs = len(in_names), len(out_avals)
    in_names_all = in_names + out_names + (
        [partition_name] if partition_name else [])
    donate = tuple(range(n_params, n_params + n_outs))

    def _body(*args):
        operands = list(args)
        if partition_name is not None:
            operands.append(partition_id_tensor())
        outs = _bass_exec_p.bind(
            *operands, out_avals=tuple(out_avals),
            in_names=tuple(in_names_all), out_names=tuple(out_names),
            lowering_input_output_aliases=(), sim_require_finite=True,
            sim_require_nnan=True, nc=nc)
        return tuple(outs)

    devices = jax.devices()[:n_cores]
    mesh = Mesh(np.asarray(devices), ("core",))
    in_specs = (PartitionSpec("core"),) * (n_params + n_outs)
    out_specs = (PartitionSpec("core"),) * n_outs
    sharded = jax.jit(
        shard_map(_body, mesh=mesh, in_specs=in_specs, out_specs=out_specs,
                  check_rep=False),
        donate_argnums=donate, keep_unused=True)
    sh = NamedSharding(mesh, PartitionSpec("core"))
    dev_in = [
        jax.device_put(
            np.concatenate([np.asarray(in_maps[c][nm])
                            for c in range(n_cores)], axis=0), sh)
        for nm in in_names
    ]

    def run_once():
        zeros = [
            jax.device_put(
                np.zeros((n_cores * z.shape[0], *z.shape[1:]), z.dtype), sh)
            for z in zero_outs
        ]
        outs = sharded(*dev_in, *zeros)
        jax.block_until_ready(outs)
        return outs, out_names

    return run_once


def kernel(combined_hm_preds, combined_lb_preds, heatmaps, labels):
    in_maps = make_in_maps(combined_hm_preds, combined_lb_preds, heatmaps,
                           labels)
    res = run(in_maps).results
    combined_loss = np.concatenate(
        [r["hm_out"].reshape(B_LOC, S, NHALF).sum(-1) for r in res], axis=0)
    labels_loss = np.concatenate([r["lb_out"] for r in res], axis=0)
    return combined_loss.astype(np.float32), labels_loss.astype(np.float32)

